# revision 19
# baseline (speedup 1.0000x reference)
"""Trainium2 Bass kernel for nn_GeneralNetworkedAE (gnn_message_passing).

Computation (per batch row b):
    features = concat(x, u)                  # [1024]
    g[a]     = features[in_idx[a]]           # [32, 128]   gather
    h[a]     = relu(g[a] @ W1[a] + b1[a])    # [32, 256]
    o[a]     = h[a] @ W2[a] + b2[a]          # [32, 28]
    out      = scatter of o by out_idx into the 896 state slots

Strategy: data-parallel over batch across 8 NeuronCores (Bs=2048 each).
The gather indices are inputs known on the host before compile, so the
gather runs on the host.  On the device everything flows transposed
(feature dims on SBUF partitions, batch on the free dim) so the matmul
keeps weights stationary:
    mm1: psum[H-chunk 128, batch 512] = W1chunk.T-free @ gT        x2 chunks
    relu+b1 fused into the PSUM->SBUF copy (greedy DVE/ACT balance)
    mm2: col-tiled: 4 agents packed in the PE array (tile_position),
         DOUT padded 28->32 so all 128 psum partitions are written.
    b2 fused into the o PSUM->SBUF copy; output stored bf16 as
    oT [A*32, Bs] (pad rows junk).  Host re-transposes, upcasts to f32
    and applies the out_idx scatter.

The kernel is PSUM-evacuation bound on TRN2 (fp32 PSUM reads are 1
elem/cycle/lane on both DVE and ACT; GPSIMD/DMA have no PSUM port), so
the copy work is split between DVE and ACT by projected busy time and
the output DMA runs in bf16 to keep HBM traffic off the critical path.
"""

import numpy as np
import ml_dtypes

import concourse.bacc as bacc
import concourse.tile as tile
from concourse import mybir
from concourse.bass_utils import run_bass_kernel_spmd

BF16 = ml_dtypes.bfloat16

B, NX, NU = 16384, 896, 128
A, DIN, H, DOUT = 32, 128, 256, 28
DOUTP = 32            # padded per-agent output width (zero cols 28..31)
N_CORES = 8
BS = B // N_CORES     # 2048 batch rows per core
BT = 512              # matmul moving free dim / psum bank
NT = BS // BT         # 4 batch tiles
NG = A // 4           # 8 groups of 4 agents (col-tiling pack)

F32 = mybir.dt.float32
BF = mybir.dt.bfloat16

N_WARMUP_MM = 36      # dummy matmuls issued under the initial DMA window
MM2_SKIP = False      # diagnostic: drop mm2+o-copies (timing-only builds)


class CopyBalancer:
    """Greedy PSUM->SBUF copy assignment by projected engine busy-ns."""

    def __init__(self, nc):
        self.nc = nc
        self.busy = {"dve": 0.0, "act": 0.0}

    def emit(self, out, psum, bias_col, relu, fd):
        cost_dve = (120 + fd) / 0.96
        cost_act = (222 + fd) / 1.2
        if self.busy["dve"] + cost_dve <= self.busy["act"] + cost_act:
            self.busy["dve"] += cost_dve
            if relu:
                self.nc.vector.tensor_scalar(
                    out=out, in0=psum, scalar1=bias_col, scalar2=0.0,
                    op0=mybir.AluOpType.add, op1=mybir.AluOpType.max)
            else:
                self.nc.vector.tensor_scalar(
                    out=out, in0=psum, scalar1=bias_col, scalar2=None,
                    op0=mybir.AluOpType.add)
        else:
            self.busy["act"] += cost_act
            self.nc.scalar.activation(
                out=out, in_=psum,
                func=(mybir.ActivationFunctionType.Relu if relu
                      else mybir.ActivationFunctionType.Identity),
                bias=bias_col, scale=1.0)


def build_program(repeat: int = 1):
    nc = bacc.Bacc(trn_type="TRN2", target_bir_lowering=False, debug=False,
                   enable_asserts=True)
    gT = nc.dram_tensor("gT", [A, DIN, BS], BF, kind="ExternalInput").ap()
    w1 = nc.dram_tensor("w1", [DIN, A * H], BF, kind="ExternalInput").ap()
    w2 = nc.dram_tensor("w2", [128, A * 2 * DOUTP], BF, kind="ExternalInput").ap()
    b1t = nc.dram_tensor("b1t", [128, A * 2], F32, kind="ExternalInput").ap()
    b2t = nc.dram_tensor("b2t", [128, NG], F32, kind="ExternalInput").ap()
    # padded rows: agent a occupies rows a*32..a*32+28; gap rows are junk
    # (discarded on the host) so each group stores as one [128, BS] DMA
    outT = nc.dram_tensor("outT", [A * DOUTP, BS], BF, kind="ExternalOutput").ap()

    with tile.TileContext(nc) as tc:
        with (
            tc.tile_pool(name="wpool", bufs=1) as wpool,
            tc.tile_pool(name="gpool", bufs=2) as gpool,
            tc.tile_pool(name="hpool", bufs=18) as hpool,
            tc.tile_pool(name="opool", bufs=2) as opool,
            tc.tile_pool(name="hpsum", bufs=3, space="PSUM") as hpsum,
            tc.tile_pool(name="opsum", bufs=2, space="PSUM") as opsum,
        ):
            w1_head = wpool.tile([DIN, 4 * H], BF)
            nc.sync.dma_start(out=w1_head[:], in_=w1[:, :4 * H])
            w1_tail = wpool.tile([DIN, (A - 4) * H], BF)
            nc.gpsimd.dma_start(out=w1_tail[:], in_=w1[:, 4 * H:])

            def w1_slice(a, m):
                if a < 4:
                    return w1_head[:, a * H + m * 128:a * H + (m + 1) * 128]
                b = a - 4
                return w1_tail[:, b * H + m * 128:b * H + (m + 1) * 128]
            w2_sb = wpool.tile([128, A * 2 * DOUTP], BF)
            nc.gpsimd.dma_start(out=w2_sb[:], in_=w2[:])
            b1_sb = wpool.tile([128, A * 2], F32)
            nc.gpsimd.dma_start(out=b1_sb[:], in_=b1t[:])
            b2_sb = wpool.tile([128, NG], F32)
            nc.gpsimd.dma_start(out=b2_sb[:], in_=b2t[:])

            # PE warm-up: dummy matmuls under the initial DMA window so the
            # HAM clock gate is released before the first real mm1 issues.
            if N_WARMUP_MM:
                wu_sb = wpool.tile([128, 128], BF)
                nc.vector.memset(wu_sb[:], 0.0)
                wu_ps = opsum.tile([64, 128], F32, tag="po")
                for _ in range(N_WARMUP_MM):
                    nc.tensor.matmul(wu_ps[:], lhsT=wu_sb[:, :64],
                                     rhs=wu_sb[:], start=True, stop=True)

            bal = CopyBalancer(nc)
            pending = None  # one-step software pipeline: mm2 lags mm1 by one T

            def emit_mm2(p):
                # m-outer / tt-inner: each 4-agent W2 column set stays
                # stationary in the PE array across both batch halves,
                # halving mm2 LDWEIGHTS traffic
                ntt = p["w"] // BT
                ps_os = []
                for _ in range(ntt):
                    ps_o = opsum.tile([128, BT], F32, tag="po")
                    ps_os.append(ps_o)
                for m in range(2):
                    for tt in range(ntt):
                        for j in range(4):
                            a = 4 * p["g"] + j
                            nc.tensor.matmul(
                                ps_os[tt][32 * j:32 * j + DOUTP, :],
                                lhsT=w2_sb[:, (a * 2 + m) * DOUTP:
                                           (a * 2 + m + 1) * DOUTP],
                                rhs=p["hts"][(j, m)][:, tt * BT:(tt + 1) * BT],
                                start=(m == 0), stop=(m == 1),
                                tile_position=(0, 32 * j),
                                skip_group_check=True,
                            )
                for tt in range(ntt):
                    c = p["c0"] + tt * BT
                    bal.emit(out=p["ostage"][:, c:c + BT],
                             psum=ps_os[tt][:],
                             bias_col=b2_sb[:, p["g"]:p["g"] + 1],
                             relu=False, fd=BT)
                # store this unit's columns immediately: overlaps the next
                # unit's compute and shortens the end-of-kernel drain tail
                c0, w = p["c0"], p["w"]
                nc.gpsimd.dma_start(
                    out=outT[p["g"] * 128:(p["g"] + 1) * 128, c0:c0 + w],
                    in_=p["ostage"][:, c0:c0 + w])

            for _r in range(repeat):
                gt8 = None
                first_special = (_r == 0)
                for g in range(NG):
                    if g == 0 and first_special:
                        # per-agent loads on the second HWDGE ring (scalar)
                        # so they run parallel to w1_head on sync: first mm1
                        # starts after max(512KB, 512KB) instead of the sum
                        gts = []
                        for j in range(4):
                            g1 = wpool.tile([DIN, BS], BF, tag=f"g0a{j}")
                            nc.scalar.dma_start(out=g1[:], in_=gT[j])
                            gts.append(g1[:, :])
                    else:
                        # 4MB DMAs load 8 agents (2 groups) at a time:
                        # SBUF [128, 8*BS] with agent-major free layout
                        start = 1 if first_special else 0
                        if (g - start) % 2 == 0:
                            na = min(8, (NG - g) * 4)
                            gt8 = gpool.tile([DIN, 8 * BS], BF, tag="gt")
                            nc.sync.dma_start(
                                out=gt8[:, :na * BS].rearrange(
                                    "p (k c) -> p k c", k=na),
                                in_=gT[4 * g:4 * g + na].rearrange(
                                    "k p c -> p k c"))
                            goff = 0
                        else:
                            goff = 4
                        gts = [gt8[:, (goff + j) * BS:(goff + j + 1) * BS]
                               for j in range(4)]
                    ostage = opool.tile([128, BS], BF, tag="ostage")
                    # batch-column units; the very last unit of the build is
                    # split in half so the pipeline tail drains finer-grained
                    units = [(0, 2 * BT), (2 * BT, 2 * BT)]
                    if _r == repeat - 1 and g == NG - 1:
                        units = [(0, 2 * BT), (2 * BT, BT), (3 * BT, BT)]
                    for ui, (c0, w) in enumerate(units):
                        hts = {}
                        for j in range(4):
                            a = 4 * g + j
                            for m in range(2):
                                ps_h = hpsum.tile([128, w], F32, tag="ph")
                                for tt in range(w // BT):
                                    c = c0 + tt * BT
                                    nc.tensor.matmul(
                                        ps_h[:, tt * BT:(tt + 1) * BT],
                                        lhsT=w1_slice(a, m),
                                        rhs=gts[j][:, c:c + BT],
                                        start=True, stop=True,
                                    )
                                h_sb = hpool.tile([128, w], BF, tag="h")
                                bal.emit(out=h_sb[:], psum=ps_h[:],
                                         bias_col=b1_sb[:, a * 2 + m:
                                                        a * 2 + m + 1],
                                         relu=True, fd=w)
                                hts[(j, m)] = h_sb
                        if pending is not None and not MM2_SKIP:
                            emit_mm2(pending)
                        pending = {"g": g, "c0": c0, "w": w,
                                   "ostage": ostage, "hts": hts,
                                   "last": ui == len(units) - 1}
                        if MM2_SKIP and ui == len(units) - 1:
                            nc.gpsimd.dma_start(
                                out=outT[g * 128:(g + 1) * 128, :],
                                in_=ostage[:])
            if pending is not None and not MM2_SKIP:
                emit_mm2(pending)
                pending = None
    nc.compile()
    return nc


def prep_inputs(x, u, W1, b1, W2, b2, in_idx):
    """Host-side shard + layout prep. Returns per-core in_maps."""
    feats = np.concatenate([np.asarray(x, np.float32),
                            np.asarray(u, np.float32)], axis=1)  # [B, 1024]
    featsT = np.ascontiguousarray(feats.T).astype(BF16)          # [1024, B]
    flat_idx = np.asarray(in_idx).reshape(-1).astype(np.int64)
    gT_full = featsT[flat_idx]                                    # [A*DIN, B]

    w1h = np.asarray(W1, np.float32).transpose(1, 0, 2).reshape(DIN, A * H)
    w1h = np.ascontiguousarray(w1h).astype(BF16)
    w2p = np.zeros((A, H, DOUTP), np.float32)
    w2p[:, :, :DOUT] = np.asarray(W2, np.float32)
    w2h = (w2p.reshape(A, 2, 128, DOUTP).transpose(2, 0, 1, 3)
           .reshape(128, A * 2 * DOUTP))
    w2h = np.ascontiguousarray(w2h).astype(BF16)
    b1h = np.ascontiguousarray(
        np.asarray(b1, np.float32).reshape(A, 2, 128).transpose(2, 0, 1)
        .reshape(128, A * 2))
    b2h = np.zeros((128, NG), np.float32)
    for g in range(NG):
        for j in range(4):
            b2h[32 * j:32 * j + DOUT, g] = np.asarray(b2, np.float32)[4 * g + j]

    in_maps = []
    for c in range(N_CORES):
        gT_c = np.ascontiguousarray(
            gT_full[:, c * BS:(c + 1) * BS]).reshape(A, DIN, BS)
        in_maps.append({"gT": gT_c, "w1": w1h, "w2": w2h,
                        "b1t": b1h, "b2t": b2h})
    return in_maps


def assemble_output(results, x, u, out_idx):
    """Gather per-core oT outputs, un-transpose, apply out_idx scatter."""
    o_pad = np.concatenate(
        [np.asarray(results[c]["outT"], dtype=np.float32)
         for c in range(N_CORES)], axis=1)                # [A*32, B]
    o_rows = o_pad.reshape(A, DOUTP, B)[:, :DOUT, :].reshape(A * DOUT, B)
    o_flat = np.ascontiguousarray(o_rows.T)               # [B, 896]
    oi = np.asarray(out_idx).reshape(-1).astype(np.int64)
    if np.array_equal(oi, np.arange(A * DOUT)):
        return o_flat
    # general scatter path (matches reference semantics)
    feats = np.concatenate([np.asarray(x, np.float32),
                            np.asarray(u, np.float32)], axis=1)
    feats[:, oi] = o_flat
    return np.ascontiguousarray(feats[:, :NX])


def kernel(x, u, W1, b1, W2, b2, in_idx, out_idx):
    nc = build_program(repeat=1)
    in_maps = prep_inputs(x, u, W1, b1, W2, b2, in_idx)
    res = run_bass_kernel_spmd(nc, in_maps, core_ids=list(range(N_CORES)))
    return assemble_output(res.results, x, u, out_idx)
